# revision 21
# baseline (speedup 1.0000x reference)
"""Trainium2 Bass kernel for nn_Agent_74844100100112 (energy-based policy sampler).

Strategy:
 - Host (inside kernel()): reproduce the jax PRNG noise exactly (fixed key 42),
   fold stepsize/noise_scale/We3 into weights & offsets, shard batch over 8 cores
   (512 batch rows x 11 chains = 5632 rows/core).
 - Device: 10 Langevin steps; per step, per chain-chunk of 512 rows (bf16):
     fwd: pre1 = [s;1] @ [We1s;be1] + a @ We1a (K=65 + K=8 matmuls), tanh;
          pre2 = h1 @ We2 + be2 (bias via ACT), tanh
     bwd: sq = h^2 (tt), d = 1-sq (ts), DH1 = We2R @ d2 (matmul, We3 folded),
          g1 = d1*dh1 (tt), GA = -(s/2)*We1a @ g1 (stepsize folded in weights)
     GA col-tiled: 4 chunks -> one PSUM bank; one clip per quad:
          u = clip(GA, +-s/2);  A = clip(A + u - OFF, -1, 1)  (fp32 quads)
 - Final energy pass in fp32r, critic in fp32.
 - Host: logsumexp over 10 sampler chains, log_prob, entropy, assemble.
"""

import numpy as np
import ml_dtypes

import concourse.bass as bass
from concourse import bacc
import concourse.mybir as mybir
import concourse.tile as tile
from concourse import bass_utils

AFT = mybir.ActivationFunctionType
ALU = mybir.AluOpType
F32 = mybir.dt.float32
F32R = mybir.dt.float32r
BF16 = mybir.dt.bfloat16

C, B, ACTD, OBS, HID = 11, 4096, 8, 64, 256
NSTEP = 10
NCORE = 8
BC = B // NCORE          # 512 batch rows per core
NCHUNK = C               # chunk == chain (512 rows each)
NQ = 3                   # quads of 4 chunks

STEP_INIT = np.float32(0.1)
STEP_FINAL = np.float32(0.1 * 0.1)


def _stepsizes():
    out = []
    for t in range(NSTEP):
        s = (STEP_INIT - STEP_FINAL) * (np.float32(1.0) - np.float32(t) / np.float32(9.0)) ** 2 + STEP_FINAL
        out.append(np.float32(s))
    return out


# ---------------------------------------------------------------- device build

_CACHE = {}


def _build():
    if "nc" in _CACHE:
        return _CACHE["nc"]

    nc = bacc.Bacc(trn_type="TRN2")
    steps = _stepsizes()

    # ---- DRAM I/O. Constants packed per dtype into single tensors:
    # wpb (bf16) cols: we1sa[0:256] (73 rows: We1s;be1;We1a) we2k0[512:768] we2k1[768:1024]
    #                  we2rT0[1024:1280] we2rT1[1280:1536] waTs[1536:1696] st_b[1696:2208]
    # wpf (f32)  cols: st_f[0:512] wc1[512:768] wc2k[768:1280] wc3k[1280:1792]
    #                  wc4[1792:1794] bc2[1794:1796] bc3[1796:1798] be2[1798:1800]
    # wpr (f32r) cols: st_r[0:512] we1s_f[512:768] we1a_f[768:1024] we2_fk[1024:1536] we3[1536:1538]
    WATS_W = NSTEP * 2 * 32
    WPB_N, WPF_N, WPR_N = 6 * HID + WATS_W + BC, 1800, 1538
    d_wpb = nc.dram_tensor("wpb", [128, WPB_N], BF16, kind="ExternalInput")
    d_wpf = nc.dram_tensor("wpf", [128, WPF_N], F32, kind="ExternalInput")
    d_wpr = nc.dram_tensor("wpr", [128, WPR_N], F32R, kind="ExternalInput")
    d_aq0 = nc.dram_tensor("aq0", [NQ, 128, BC], F32, kind="ExternalInput")
    d_offq = nc.dram_tensor("offq", [NSTEP, 128, NQ * BC], F32, kind="ExternalInput")

    d_a_out = nc.dram_tensor("a_out", [ACTD, BC], F32, kind="ExternalOutput")
    d_e_out = nc.dram_tensor("e_out", [1, C * BC], F32, kind="ExternalOutput")
    d_v_out = nc.dram_tensor("v_out", [1, BC], F32, kind="ExternalOutput")

    with tile.TileContext(nc) as tc:
        with (
            tc.tile_pool(name="const", bufs=1) as cp,
            tc.tile_pool(name="work", bufs=2) as sb,
            tc.tile_pool(name="psum", bufs=1, space="PSUM") as pp,
        ):
            # ---- persistent tiles (packed constants + live state)
            wpb = cp.tile([128, WPB_N], BF16)
            wpf = cp.tile([128, WPF_N], F32)
            wpr = cp.tile([128, WPR_N], F32R)
            we1sa_b = wpb[:, 0:HID]
            we2_b = [wpb[:, 2 * HID + k * HID:2 * HID + (k + 1) * HID] for k in range(2)]
            we2rT_b = [wpb[:, 4 * HID + k * HID:4 * HID + (k + 1) * HID] for k in range(2)]
            waTs_b = wpb[:, 6 * HID:6 * HID + WATS_W]
            st_b = wpb[0:96, 6 * HID + WATS_W:6 * HID + WATS_W + BC]
            st_f = wpf[0:OBS + 1, 0:BC]
            wc1_f = wpf[0:OBS + 1, BC:BC + HID]
            wc2_f = [wpf[:, 768 + k * HID:768 + (k + 1) * HID] for k in range(2)]
            wc3_f = [wpf[:, 1280 + k * HID:1280 + (k + 1) * HID] for k in range(2)]
            wc4_f = wpf[:, 1792:1794]
            bc2 = wpf[:, 1794:1796]
            bc3 = wpf[:, 1796:1798]
            be2 = wpf[:, 1798:1800]
            st_r = wpr[0:OBS + 1, 0:BC]
            we1s_f = wpr[0:OBS + 1, BC:BC + HID]
            we1a_f = wpr[:, 768:1024]
            we2_f = [wpr[:, 1024 + k * HID:1024 + (k + 1) * HID] for k in range(2)]
            we3_f = wpr[:, 1536:1538]
            aq = [cp.tile([128, BC], F32, tag=f"aq{q}", name=f"aq{q}") for q in range(NQ)]
            xt = [cp.tile([128, BC], BF16, tag=f"xt{c}", name=f"xt{c}") for c in range(NCHUNK)]
            uq = [cp.tile([128, BC], F32, tag=f"uq{q}", name=f"uq{q}") for q in range(NQ)]
            af = [cp.tile([128, BC], F32R, tag=f"af{q}", name=f"af{q}") for q in range(NQ)]
            e_sb = cp.tile([1, C * BC], F32)
            v_sb = cp.tile([1, BC], F32)

            # ---- loads
            dma = nc.sync.dma_start
            dma(wpb[:], d_wpb[:])
            dma(aq[0][:], d_aq0[0])
            dma(wpf[:], d_wpf[:])
            for q in range(1, NQ):
                dma(aq[q][:], d_aq0[q])
            for q in range(NQ):
                nc.vector.memset(uq[q][:], 0.0)
            for c in range(NCHUNK):
                q, j = divmod(c, 4)
                nc.gpsimd.tensor_copy(xt[c][0:96, :], st_b)
                nc.gpsimd.tensor_copy(xt[c][96:128, :], aq[q][32 * j:32 * j + 32, :])

            def deferred_loads():
                dma(wpr[:], d_wpr[:])

            mm = nc.tensor.matmul
            act = nc.scalar.activation
            stt = nc.vector.scalar_tensor_tensor
            ts = nc.vector.tensor_scalar
            tt = nc.vector.tensor_tensor

            def final_chunk(c):
                q, j = divmod(c, 4)
                a_sl = af[q][32 * j:32 * j + 8, :]
                h1f = sb.tile([128, 2 * BC], F32R, tag="h1f", bufs=2, name="h1f")
                for h in range(2):
                    hs = slice(h * 128, (h + 1) * 128)
                    os_ = slice(h * BC, (h + 1) * BC)
                    pre1 = pp.tile([128, BC], F32, tag="pre", bufs=4, name="pre1f")
                    mm(pre1[:], we1s_f[:, hs], st_r[:], start=True, stop=False)
                    mm(pre1[:], we1a_f[32 * j:32 * j + 8, hs], a_sl,
                       start=False, stop=True, tile_position=(32 * j, 0))
                    act(h1f[:, os_], pre1[:], AFT.Tanh)
                h2f = sb.tile([128, 2 * BC], F32R, tag="h2f", bufs=2, name="h2f")
                for h in range(2):
                    hs = slice(h * 128, (h + 1) * 128)
                    os_ = slice(h * BC, (h + 1) * BC)
                    pre2 = pp.tile([128, BC], F32, tag="pre", bufs=4, name="pre2f")
                    mm(pre2[:], we2_f[0][:, hs], h1f[:, 0:BC], start=True, stop=False)
                    mm(pre2[:], we2_f[1][:, hs], h1f[:, BC:2 * BC], start=False, stop=True)
                    act(h2f[:, os_], pre2[:], AFT.Tanh, bias=be2[:, h:h + 1])
                ev = pp.tile([1, BC], F32, tag="ga", bufs=2, name="ev")
                for k in range(2):
                    mm(ev[:], we3_f[:, k:k + 1], h2f[:, k * BC:(k + 1) * BC],
                       start=(k == 0), stop=(k == 1))
                nc.scalar.copy(e_sb[0:1, c * BC:(c + 1) * BC], ev[:])

            # ---- critic layers as functions, emitted spread across steps 0-3
            ct = {}

            def critic_layer1():
                t1 = sb.tile([128, 2 * BC], F32, tag="ct1", bufs=1)
                for h in range(2):
                    hs = slice(h * 128, (h + 1) * 128)
                    os_ = slice(h * BC, (h + 1) * BC)
                    cpre = pp.tile([128, BC], F32, tag="pre", bufs=4, name="cpre")
                    mm(cpre[:], wc1_f[:, hs], st_f[:], start=True, stop=True)
                    act(t1[:, os_], cpre[:], AFT.Tanh)
                ct["t1"] = t1

            def critic_layer2():
                t1 = ct["t1"]
                t2 = sb.tile([128, 2 * BC], F32, tag="ct2", bufs=1)
                for h in range(2):
                    hs = slice(h * 128, (h + 1) * 128)
                    os_ = slice(h * BC, (h + 1) * BC)
                    cpre2 = pp.tile([128, BC], F32, tag="pre", bufs=4, name="cpre2")
                    mm(cpre2[:], wc2_f[0][:, hs], t1[:, 0:BC], start=True, stop=False)
                    mm(cpre2[:], wc2_f[1][:, hs], t1[:, BC:2 * BC], start=False, stop=True)
                    act(t2[:, os_], cpre2[:], AFT.Tanh, bias=bc2[:, h:h + 1])
                ct["t2"] = t2

            def critic_layer3():
                t2 = ct["t2"]
                t3 = sb.tile([128, 2 * BC], F32, tag="ct3", bufs=1)
                for h in range(2):
                    hs = slice(h * 128, (h + 1) * 128)
                    os_ = slice(h * BC, (h + 1) * BC)
                    cpre3 = pp.tile([128, BC], F32, tag="pre", bufs=4, name="cpre3")
                    mm(cpre3[:], wc3_f[0][:, hs], t2[:, 0:BC], start=True, stop=False)
                    mm(cpre3[:], wc3_f[1][:, hs], t2[:, BC:2 * BC], start=False, stop=True)
                    act(t3[:, os_], cpre3[:], AFT.Tanh, bias=bc3[:, h:h + 1])
                ct["t3"] = t3

            def critic_layer4():
                t3 = ct["t3"]
                vv = pp.tile([1, BC], F32, tag="ga", bufs=2, name="vv")
                for k in range(2):
                    mm(vv[:], wc4_f[:, k:k + 1], t3[:, k * BC:(k + 1) * BC],
                       start=(k == 0), stop=(k == 1))
                nc.scalar.copy(v_sb[:], vv[:])

            critic_layers = [critic_layer1, critic_layer2, critic_layer3, critic_layer4]

            # ================= MCMC loop =================
            for t in range(NSTEP):
                if t == NSTEP - 2:
                    deferred_loads()
                s2 = float(steps[t]) / 2.0
                off = sb.tile([128, NQ * BC], F32, tag="off", bufs=2)
                dma(off[:], d_offq[t])
                offt = [off[:, q * BC:(q + 1) * BC] for q in range(NQ)]
                gaq = None
                for c in range(NCHUNK):
                    if c == 5 and t < len(critic_layers):
                        critic_layers[t]()
                    q, j = divmod(c, 4)
                    h1 = sb.tile([128, 2 * BC], BF16, tag="h1", bufs=3)
                    for h in range(2):
                        hs = slice(h * 128, (h + 1) * 128)
                        os_ = slice(h * BC, (h + 1) * BC)
                        pre1 = pp.tile([128, BC], F32, tag="pre", bufs=4, name="pre1")
                        mm(pre1[:], we1sa_b[:, hs], xt[c][:], start=True, stop=True)
                        act(h1[:, os_], pre1[:], AFT.Tanh)
                    h2 = sb.tile([128, 2 * BC], BF16, tag="h2", bufs=3)
                    for h in range(2):
                        hs = slice(h * 128, (h + 1) * 128)
                        os_ = slice(h * BC, (h + 1) * BC)
                        pre2 = pp.tile([128, BC], F32, tag="pre", bufs=4, name="pre2")
                        mm(pre2[:], we2_b[0][:, hs], h1[:, 0:BC], start=True, stop=False)
                        mm(pre2[:], we2_b[1][:, hs], h1[:, BC:2 * BC], start=False, stop=True)
                        act(h2[:, os_], pre2[:], AFT.Tanh, bias=be2[:, h:h + 1])
                    sq2 = sb.tile([128, 2 * BC], BF16, tag="sq", bufs=4)
                    if c % 3 == 1:
                        act(sq2[:], h2[:], AFT.Square)
                    else:
                        tt(sq2[:], h2[:], h2[:], op=ALU.mult)
                    d2 = sb.tile([128, 2 * BC], BF16, tag="d", bufs=4)
                    ts(d2[:], sq2[:], -1.0, 1.0, op0=ALU.mult, op1=ALU.add)
                    sq1 = sb.tile([128, 2 * BC], BF16, tag="sq", bufs=4)
                    tt(sq1[:], h1[:], h1[:], op=ALU.mult)
                    g1 = sb.tile([128, 2 * BC], BF16, tag="g1", bufs=3)
                    for h in range(2):
                        hs = slice(h * 128, (h + 1) * 128)
                        os_ = slice(h * BC, (h + 1) * BC)
                        dh1 = pp.tile([128, BC], F32, tag="dh1", bufs=2, name="dh1")
                        mm(dh1[:], we2rT_b[0][:, hs], d2[:, 0:BC], start=True, stop=False)
                        mm(dh1[:], we2rT_b[1][:, hs], d2[:, BC:2 * BC], start=False, stop=True)
                        amracc = sb.tile([128, 1], F32, tag="amracc", bufs=4, name="amracc")
                        nc.vector.affine_mul_reduce(g1[:, os_], amracc[:], sq1[:, os_], dh1[:],
                                                    scale=-1.0, bias=1.0)
                    if j == 0:
                        gaq = pp.tile([128, BC], F32, tag="ga", bufs=2)
                    for k in range(2):
                        ws = slice((t * 2 + k) * 32, (t * 2 + k + 1) * 32)
                        mm(gaq[32 * j:32 * j + 32, :], waTs_b[:, ws], g1[:, k * BC:(k + 1) * BC],
                           start=(k == 0), stop=(k == 1), tile_position=(0, 32 * j))
                    if j == 3 or c == NCHUNK - 1:
                        # quad complete: clip + update + bf16 refresh
                        ts(uq[q][:], gaq[:], s2, -s2, op0=ALU.min, op1=ALU.max)
                        x = sb.tile([128, BC], F32, tag="x", bufs=3)
                        stt(x[:], offt[q][:], -1.0, uq[q][:], op0=ALU.mult, op1=ALU.add)
                        tt(aq[q][:], aq[q][:], x[:], op=ALU.add)
                        ts(aq[q][:], aq[q][:], 1.0, -1.0, op0=ALU.min, op1=ALU.max)
                        if t < NSTEP - 1:
                            for cz in range(4 * q, min(4 * q + 4, NCHUNK)):
                                jz = cz % 4
                                nc.gpsimd.tensor_copy(xt[cz][96:128, :],
                                                      aq[q][32 * jz:32 * jz + 32, :])
                        else:
                            # last step: af ready per quad so final pass can overlap
                            nc.scalar.copy(af[q][:], aq[q][:])

            # ================= final energy pass (fp32r) =================
            for cf in range(NCHUNK):
                final_chunk(cf)

            # ================= outputs =================
            dma(d_a_out[:], aq[0][0:8, :])
            dma(d_e_out[:], e_sb[:])
            dma(d_v_out[:], v_sb[:])

    nc.finalize()
    _CACHE["nc"] = nc
    return nc


# ---------------------------------------------------------------- host side

def _host_noise(temperature):
    import jax
    import jax.numpy as jnp
    cpu = jax.devices("cpu")[0]
    with jax.default_device(cpu):
        T = np.float32(np.exp(np.float32(temperature[0])))
        key = jax.random.key(42)
        k_init, k_loop = jax.random.split(key)
        a0 = np.asarray(jnp.tanh(jax.random.normal(k_init, (C, B, ACTD))))
        noises = []
        k = k_loop
        for _ in range(NSTEP):
            k, kn = jax.random.split(k)
            noises.append(np.asarray(jax.random.normal(kn, (C, B, ACTD))))
    steps = _stepsizes()
    off = [np.float32(steps[t] * T) * noises[t] for t in range(NSTEP)]
    return a0, off, steps, T


def _to_quads(arr):
    """arr [C, BCrows, ACTD] -> [NQ, 128, BC] (chunk j of quad at partitions 32j..32j+8)."""
    out = np.zeros((NQ, 128, BC), np.float32)
    for i in range(C):
        q, j = divmod(i, 4)
        out[q, 32 * j:32 * j + 8, :] = arr[i].T
    return out


def _prep_in_maps(inputs):
    f32 = lambda x: np.ascontiguousarray(np.asarray(x, np.float32))
    b16 = lambda x: np.ascontiguousarray(np.asarray(x, np.float32).astype(ml_dtypes.bfloat16))
    states = f32(inputs["states"])
    We1 = f32(inputs["We1"]); be1 = f32(inputs["be1"])
    We2 = f32(inputs["We2"]); be2 = f32(inputs["be2"])
    We3 = f32(inputs["We3"]); be3 = f32(inputs["be3"])
    Wc1 = f32(inputs["Wc1"]); bc1 = f32(inputs["bc1"])
    Wc2 = f32(inputs["Wc2"]); bc2 = f32(inputs["bc2"])
    Wc3 = f32(inputs["Wc3"]); bc3 = f32(inputs["bc3"])
    Wc4 = f32(inputs["Wc4"]); bc4 = f32(inputs["bc4"])
    temperature = f32(inputs["temperature"])

    a0, off, steps, T = _host_noise(temperature)

    we3v = We3[:, 0]
    we2rT = we3v[:, None] * We2.T                       # [256,256] lhsT for DH1
    waTs = np.zeros((128, NSTEP * 2 * 32), np.float32)
    for t in range(NSTEP):
        w = np.float32(-(steps[t] / np.float32(2.0))) * We1[OBS:].T    # [256, 8]
        for k in range(2):
            waTs[:, (t * 2 + k) * 32:(t * 2 + k) * 32 + ACTD] = w[k * 128:(k + 1) * 128]

    we1a4 = np.zeros((128, HID), np.float32)
    for j in range(4):
        we1a4[32 * j:32 * j + ACTD] = We1[OBS:]

    we1sb = np.concatenate([We1[:OBS], be1[None, :]], axis=0)      # [65, 256]
    wc1b = np.concatenate([Wc1, bc1[None, :]], axis=0)             # [65, 256]

    pack2 = lambda v: np.stack([v[:128], v[128:]], axis=1)       # [256] -> [128, 2]

    def pad128(a):
        out = np.zeros((128, a.shape[1]), np.float32)
        out[:a.shape[0]] = a
        return out

    # bf16 pack (st_b appended per core below): [128, 2208]
    we1sab = np.zeros((128, HID), np.float32)   # rows: 0-63 We1s, 64 be1, 96-103 We1a
    we1sab[0:OBS + 1] = we1sb
    we1sab[96:96 + ACTD] = We1[OBS:]
    wpb_base = np.concatenate([
        we1sab, np.zeros((128, HID), np.float32), pad128(We2[:128]), pad128(We2[128:]),
        pad128(we2rT[:128]), pad128(we2rT[128:]), waTs,
    ], axis=1)
    # f32 pack: st_f per core + critic weights/biases: [128, 1800]
    wpf_tail = np.concatenate([
        pad128(wc1b), pad128(Wc2[:128]), pad128(Wc2[128:]),
        pad128(Wc3[:128]), pad128(Wc3[128:]),
        pack2(Wc4[:, 0]), pack2(bc2), pack2(bc3), pack2(be2),
    ], axis=1)
    # f32r pack: st_r per core + energy weights: [128, 1538]
    wpr_tail = np.concatenate([
        pad128(we1sb), we1a4, pad128(We2[:128]), pad128(We2[128:]),
        pack2(We3[:, 0]),
    ], axis=1)

    in_maps = []
    for cc in range(NCORE):
        bs = slice(cc * BC, (cc + 1) * BC)
        st = pad128(np.concatenate([states[bs].T, np.ones((1, BC), np.float32)], axis=0))
        aq0 = _to_quads(a0[:, bs, :])
        offq = np.ascontiguousarray(np.stack(
            [_to_quads(off[t][:, bs, :]).transpose(1, 0, 2).reshape(128, NQ * BC)
             for t in range(NSTEP)]))
        m = {
            "wpb": b16(np.concatenate([wpb_base, st], axis=1)),
            "wpf": np.ascontiguousarray(np.concatenate([st, wpf_tail], axis=1)),
            "wpr": np.ascontiguousarray(np.concatenate([st, wpr_tail], axis=1)),
            "aq0": aq0, "offq": offq,
        }
        in_maps.append(m)

    meta = {"T": T, "be3": np.float32(be3[0]), "bc4": np.float32(bc4[0])}
    return in_maps, meta


def _postprocess(results, meta):
    T = meta["T"]
    actions = np.zeros((B, ACTD), np.float32)
    E = np.zeros((C, B), np.float32)
    V = np.zeros((B, 1), np.float32)
    for cc, r in enumerate(results):
        bs = slice(cc * BC, (cc + 1) * BC)
        actions[bs] = r["a_out"].T
        E[:, bs] = r["e_out"].reshape(C, BC)
        V[bs, 0] = r["v_out"][0]
    E += meta["be3"]
    V += meta["bc4"]
    energy = E[0]
    neg = -E[1:] / T
    m = neg.max(axis=0)
    lse = m + np.log(np.exp(neg - m).sum(axis=0))
    log_prob = (-energy / T - lse).astype(np.float32)
    entropy = np.float32(T * log_prob.mean(dtype=np.float64))
    return actions, log_prob, entropy, V


def _run(inputs, **spmd_kwargs):
    nc = _build()
    in_maps, meta = _prep_in_maps(inputs)
    res = bass_utils.run_bass_kernel_spmd(nc, in_maps, core_ids=list(range(NCORE)), **spmd_kwargs)
    return _postprocess(res.results, meta), res


def kernel(**inputs):
    out, _ = _run(inputs)
    return out
